# revision 23
# baseline (speedup 1.0000x reference)
"""Trainium2 Bass kernel: soft visual attention (encoder-decoder attention).

Computes, for encoder_out X (B=256, P=196, E=2048), decoder_hidden h (B, D=512):
    att1 = X @ W_enc + b_enc                      (B, P, A=512)
    att2 = h @ W_dec + b_dec                      (B, 1, A)
    e    = relu(att1 + att2) @ W_att + b_att      (B, P)
    alpha = softmax(e, axis=-1)                   (B, P)
    z    = sum_p alpha[b,p] * X[b,p,:]            (B, E)
returns (z, alpha).

Strategy: data-parallel over batch across 8 NeuronCores (32 batches each).
Per core, batches are processed in groups of 4 (784 rows), with the row
(batch*pixel) dimension flattened and tiled by 128.  The heavy matmul runs in
bf16 with a "transposed activations" dataflow: X is cast to bf16 on load
(SWDGE cast-DMA), transposed on-chip via the DMA xbar so the contraction dim
(enc) lies on partitions, and att1^T (att on partitions, rows on free dim) is
accumulated in PSUM.  relu(att1+att2) is a single scalar-engine activation
per batch segment with att2^T as a per-partition bias; the scalar score e is
a K-contraction matmul with W_att; softmax runs batched (4,196); z is
computed as a rank-4 matmul per row tile with a mask-selected alpha matrix
(alpha scattered to flat-row layout via a second tiny xbar transpose).
Biases are folded in as K=1 rank-1 matmuls against a ones-row.
"""

import sys

sys.path.insert(0, "/opt/trn_rl_repo")

import numpy as np
from contextlib import ExitStack

import concourse.bass as bass
import concourse.tile as tile
from concourse import bacc, mybir
from concourse.bass_utils import run_bass_kernel_spmd

F32 = mybir.dt.float32
BF16 = mybir.dt.bfloat16
Act = mybir.ActivationFunctionType
Alu = mybir.AluOpType
Axis = mybir.AxisListType

B, P, E, D, A = 256, 196, 2048, 512, 512
NCORES = 8
BG = B // NCORES            # 32 batches per core
G = 4                       # batches per group
NGRP = BG // G              # 8 groups per core
GR = G * P                  # 784 rows per group
NT = 7                      # row tiles per group (6 x 128 + 1 x 16)
LAST = GR - 6 * 128         # 16
EK = E // 128               # 16 contraction k-tiles for the big matmul
AM = A // 128               # 4 attention-dim chunks
DK = D // 128               # 4 contraction k-tiles for att2
ZC = E // 512               # 4 enc chunks for z
BLOCKS = [(0, 512), (512, GR - 512)]  # PSUM column blocks of the 784 cols


def _segments():
    """Per block: (local batch, start col within block, ncols)."""
    segs = {0: [], 1: []}
    for bl in range(G):
        lo, hi = bl * P, (bl + 1) * P
        for blk, (c0, cn) in enumerate(BLOCKS):
            s, e_ = max(lo, c0), min(hi, c0 + cn)
            if s < e_:
                segs[blk].append((bl, s - c0, e_ - s))
    return segs


SEGS = _segments()


def build_nc(repeat=1):
    nc = bacc.Bacc("TRN2", target_bir_lowering=False, debug=False,
                   num_devices=NCORES)

    x = nc.dram_tensor("x", [BG * P, E], F32, kind="ExternalInput")
    h = nc.dram_tensor("h", [BG, D], F32, kind="ExternalInput")
    w_enc = nc.dram_tensor("w_enc", [E, A], F32, kind="ExternalInput")
    b_enc = nc.dram_tensor("b_enc", [A], F32, kind="ExternalInput")
    w_dec = nc.dram_tensor("w_dec", [D, A], F32, kind="ExternalInput")
    b_dec = nc.dram_tensor("b_dec", [A], F32, kind="ExternalInput")
    w_att = nc.dram_tensor("w_att", [A, 1], F32, kind="ExternalInput")
    b_att = nc.dram_tensor("b_att", [1], F32, kind="ExternalInput")

    z_out = nc.dram_tensor("z_out", [BG, E], F32, kind="ExternalOutput")
    alpha_out = nc.dram_tensor("alpha_out", [BG, P], F32, kind="ExternalOutput")

    alpha_flat = alpha_out.ap().rearrange("b p -> (b p)")

    with tile.TileContext(nc) as tc, ExitStack() as ctx:
        consts = ctx.enter_context(tc.tile_pool(name="consts", bufs=1))
        xn_pool = ctx.enter_context(tc.tile_pool(name="xn", bufs=3))

        xn_cache = {}

        def cast_load(rep, g):
            """X rows of group g: cast-load to bf16 (flat 128-row tiles)."""
            row0 = g * GR
            xn = xn_pool.tile([128, NT, E], BF16, tag="xn",
                              name=f"xn{rep}_{g}")
            nc.gpsimd.dma_start(
                out=xn[:, 0:6, :],
                in_=x.ap()[row0:row0 + 768, :].rearrange("(t p) e -> p t e",
                                                         p=128))
            nc.gpsimd.dma_start(out=xn[0:LAST, 6, :],
                                in_=x.ap()[row0 + 768:row0 + GR, :])
            xn_cache[(rep, g)] = xn

        # start streaming X for the first group before anything else
        cast_load(0, 0)

        # ---- weights (bf16, contraction dim on partitions, k-tiled) ----
        w_enc_sb = consts.tile([128, EK, A], BF16)
        w_enc_r = w_enc.ap().rearrange("(k p) a -> p k a", p=128)
        for kc in range(4):
            nc.gpsimd.dma_start(out=w_enc_sb[:, 4 * kc:4 * kc + 4, :],
                                in_=w_enc_r[:, 4 * kc:4 * kc + 4, :])
        w_dec_sb = consts.tile([128, DK, A], BF16)
        nc.gpsimd.dma_start(out=w_dec_sb,
                            in_=w_dec.ap().rearrange("(k p) a -> p k a", p=128))
        w_attT_sb = consts.tile([128, AM], BF16)
        for m in range(AM):
            nc.gpsimd.dma_start(out=w_attT_sb[:, m:m + 1],
                                in_=w_att.ap()[m * 128:(m + 1) * 128, :])
        b_enc_row = consts.tile([1, A], BF16)
        nc.gpsimd.dma_start(out=b_enc_row, in_=b_enc.ap()[None, :])
        b_dec_row = consts.tile([1, A], BF16)
        nc.gpsimd.dma_start(out=b_dec_row, in_=b_dec.ap()[None, :])
        b_att_sb = consts.tile([1, 1], BF16)
        nc.gpsimd.dma_start(out=b_att_sb, in_=b_att.ap()[None, :])
        ones_row = consts.tile([1, GR], BF16)
        nc.vector.memset(ones_row, 1.0)

        # ---- static batch-membership mask M0[p, t, b] over one group ----
        # row = 128*t + p belongs to local batch b iff 196b <= row < 196(b+1)
        m0_ones = consts.tile([128, NT, G], BF16)
        nc.vector.memset(m0_ones, 1.0)
        m0a = consts.tile([128, NT, G], BF16)
        nc.gpsimd.affine_select(out=m0a, in_=m0_ones,
                                pattern=[[128, NT], [-P, G]], base=0,
                                channel_multiplier=1,
                                compare_op=Alu.is_ge, fill=0.0)
        m0 = consts.tile([128, NT, G], BF16)
        # row < 196(b+1)  <=>  -row + 196b + 195 >= 0
        nc.gpsimd.affine_select(out=m0, in_=m0a,
                                pattern=[[-128, NT], [P, G]], base=P - 1,
                                channel_multiplier=-1,
                                compare_op=Alu.is_ge, fill=0.0)

        # ---- att2^T = (h @ W_dec + b_dec + b_enc)^T : (att on parts, batch) ----
        att2T = consts.tile([128, AM, BG], F32)
        with tc.tile_pool(name="att2ps", bufs=1, space="PSUM") as att2psum, \
             tc.tile_pool(name="prep", bufs=1) as prep:
            h_nat = prep.tile([BG, D], BF16)
            nc.gpsimd.dma_start(out=h_nat, in_=h.ap())
            hT = prep.tile([128, DK, BG], BF16)
            nc.sync.dma_start(out=hT, in_=h_nat[0:BG, :], transpose=True)
            for m in range(AM):
                ps2 = att2psum.tile([128, BG], F32, tag="att2")
                for k in range(DK):
                    nc.tensor.matmul(ps2, w_dec_sb[:, k, m * 128:(m + 1) * 128],
                                     hT[:, k, :], start=(k == 0), stop=False)
                nc.tensor.matmul(ps2, b_dec_row[0:1, m * 128:(m + 1) * 128],
                                 ones_row[0:1, 0:BG], start=False, stop=False)
                nc.tensor.matmul(ps2, b_enc_row[0:1, m * 128:(m + 1) * 128],
                                 ones_row[0:1, 0:BG], start=False, stop=True)
                nc.vector.tensor_copy(att2T[:, m, :], ps2)

        # ---- main pipeline over batch groups ----
        xt_pool = ctx.enter_context(tc.tile_pool(name="xt", bufs=2))
        r_pool = ctx.enter_context(tc.tile_pool(name="r", bufs=3))
        small = ctx.enter_context(tc.tile_pool(name="small", bufs=2))
        att1_ps = ctx.enter_context(tc.tile_pool(name="att1ps", bufs=2,
                                                 space="PSUM"))
        e_psp = ctx.enter_context(tc.tile_pool(name="eps", bufs=1,
                                               space="PSUM"))
        z_psp = ctx.enter_context(tc.tile_pool(name="zps", bufs=1,
                                               space="PSUM"))

        def z_phase(g, xn, amask):
            """z = masked-alpha^T @ X (contraction over rows, per tile)."""
            z_ps = [z_psp.tile([G, 512], F32, tag=f"z{c}", name=f"z_ps{c}")
                    for c in range(ZC)]
            for t in range(NT):
                rows = 128 if t < 6 else LAST
                for c in range(ZC):
                    nc.tensor.matmul(z_ps[c], amask[0:rows, t, :],
                                     xn[0:rows, t, c * 512:(c + 1) * 512],
                                     start=(t == 0), stop=(t == NT - 1))
            z_sb = small.tile([G, E], F32, tag="zsb", name="z_sb")
            for c in range(ZC):
                nc.vector.tensor_copy(z_sb[:, c * 512:(c + 1) * 512], z_ps[c])
            nc.sync.dma_start(out=z_out.ap()[g * G:(g + 1) * G, :], in_=z_sb)

        pending = {}
        for rep in range(repeat):
          for g in range(NGRP):
            xn = xn_cache.pop((rep, g))
            xT = xt_pool.tile([128, EK, GR], BF16, tag="xT")
            for t in range(NT):
                rows = 128 if t < 6 else LAST
                nc.sync.dma_start(out=xT[:, :, t * 128:t * 128 + rows],
                                  in_=xn[0:rows, t, :], transpose=True)
            if g + 1 < NGRP:
                cast_load(rep, g + 1)
            elif rep + 1 < repeat:
                cast_load(rep + 1, 0)

            # att1^T chunks + relu(+att2 bias) + e accumulation
            e_ps = e_psp.tile([1, 1024], F32, tag="e")
            for m in range(AM):
                r_m = r_pool.tile([128, GR], BF16, tag="R")
                for blk, (c0, cn) in enumerate(BLOCKS):
                    ps = att1_ps.tile([128, 512], F32, tag="att1")
                    for k in range(EK):
                        nc.tensor.matmul(ps[:, 0:cn],
                                         w_enc_sb[:, k, m * 128:(m + 1) * 128],
                                         xT[:, k, c0:c0 + cn],
                                         start=(k == 0), stop=(k == EK - 1))
                    for (bl, s0, sn) in SEGS[blk]:
                        nc.vector.tensor_scalar(
                            r_m[:, c0 + s0:c0 + s0 + sn], ps[:, s0:s0 + sn],
                            att2T[:, m, g * G + bl:g * G + bl + 1], 0.0,
                            Alu.add, Alu.max)
                    nc.tensor.matmul(e_ps[:, c0:c0 + cn],
                                     w_attT_sb[:, m:m + 1], r_m[:, c0:c0 + cn],
                                     start=(m == 0), stop=(m == AM - 1))
            # (b_att is a uniform shift of e; softmax is shift-invariant, so
            # it never affects alpha or z and is deliberately dropped.)

            # e (1,784) -> (4,196): same linear byte order, one SBUF-SBUF DMA
            e_row = small.tile([1, GR], F32, tag="erow")
            nc.vector.tensor_copy(e_row[:, 0:512], e_ps[:, 0:512])
            nc.vector.tensor_copy(e_row[:, 512:GR], e_ps[:, 512:GR])
            e_bt = small.tile([G, P], F32, tag="ebt")
            nc.sync.dma_start(out=e_bt, in_=e_row)
            mneg = small.tile([G, 1], F32, tag="mneg")
            nc.vector.tensor_reduce(mneg, e_bt, axis=Axis.X, op=Alu.max,
                                    negate=True)
            pexp = small.tile([G, P], F32, tag="pexp")
            ssum = small.tile([G, 1], F32, tag="ssum")
            nc.scalar.activation(pexp, e_bt, Act.Exp, bias=mneg[:, 0:1],
                                 scale=1.0, accum_out=ssum)
            rsum = small.tile([G, 1], F32, tag="rsum")
            nc.vector.reciprocal(rsum, ssum)
            alpha_g = small.tile([G, P], F32, tag="alphag")
            nc.vector.tensor_scalar_mul(alpha_g, pexp, rsum[:, 0:1])
            nc.sync.dma_start(out=alpha_out.ap()[g * G:(g + 1) * G, :],
                              in_=alpha_g)

            # alpha -> flat-row-retiled bf16 via DRAM read-back
            row0 = g * GR
            al_nat = small.tile([16, 128], BF16, tag="alnat")
            nc.vector.memset(al_nat, 0.0)
            nc.gpsimd.dma_start(
                out=al_nat[0:6, :],
                in_=alpha_flat[row0:row0 + 768].rearrange("(t q) -> t q",
                                                          q=128))
            nc.gpsimd.dma_start(out=al_nat[6:7, 0:LAST],
                                in_=alpha_flat[row0 + 768:row0 + GR][None, :])
            alT = small.tile([128, 16], BF16, tag="alT")
            nc.sync.dma_start(out=alT, in_=al_nat, transpose=True)
            amask = small.tile([128, NT, G], BF16, tag="amask")
            alT_b = bass.AP(tensor=alT.tensor, offset=alT[:, 0:NT].offset,
                            ap=[alT[:, 0:NT].ap[0], alT[:, 0:NT].ap[1],
                                [0, G]])
            nc.vector.tensor_mul(amask, alT_b, m0)

            # defer z by one group so the softmax round-trip hides behind
            # the next group's matmuls
            gi = rep * NGRP + g
            pending[gi] = (g, xn, amask)
            if gi >= 1:
                pg, pxn, pam = pending.pop(gi - 1)
                z_phase(pg, pxn, pam)
        last = repeat * NGRP - 1
        pg, pxn, pam = pending.pop(last)
        z_phase(pg, pxn, pam)

    nc.compile()
    return nc


_NC_CACHE = {}


def _get_nc():
    if "nc" not in _NC_CACHE:
        _NC_CACHE["nc"] = build_nc()
    return _NC_CACHE["nc"]


def _in_maps(encoder_out, decoder_hidden, W_enc, b_enc, W_dec, b_dec, W_att,
             b_att):
    x = np.ascontiguousarray(np.asarray(encoder_out, dtype=np.float32))
    h = np.ascontiguousarray(np.asarray(decoder_hidden, dtype=np.float32))
    shared = {
        "w_enc": np.ascontiguousarray(np.asarray(W_enc, np.float32)),
        "b_enc": np.ascontiguousarray(np.asarray(b_enc, np.float32)),
        "w_dec": np.ascontiguousarray(np.asarray(W_dec, np.float32)),
        "b_dec": np.ascontiguousarray(np.asarray(b_dec, np.float32)),
        "w_att": np.ascontiguousarray(np.asarray(W_att, np.float32)),
        "b_att": np.ascontiguousarray(np.asarray(b_att, np.float32)),
    }
    maps = []
    for i in range(NCORES):
        maps.append({
            "x": np.ascontiguousarray(
                x[i * BG:(i + 1) * BG].reshape(BG * P, E)),
            "h": np.ascontiguousarray(h[i * BG:(i + 1) * BG]),
            **shared,
        })
    return maps


def kernel(encoder_out, decoder_hidden, W_enc, b_enc, W_dec, b_dec, W_att,
           b_att, _trace=False):
    nc = _get_nc()
    maps = _in_maps(encoder_out, decoder_hidden, W_enc, b_enc, W_dec, b_dec,
                    W_att, b_att)
    res = run_bass_kernel_spmd(nc, maps, core_ids=list(range(NCORES)),
                               trace=_trace)
    z = np.concatenate([np.asarray(res.results[i]["z_out"])
                        for i in range(NCORES)], axis=0)
    alpha = np.concatenate([np.asarray(res.results[i]["alpha_out"])
                            for i in range(NCORES)], axis=0)
    if _trace:
        kernel._last_exec_time_ns = res.exec_time_ns
        kernel._last_results = res
    return (z.astype(np.float32), alpha.astype(np.float32))


# revision 46
# speedup vs baseline: 1.4953x; 1.4953x over previous
"""Trainium2 Bass kernel: soft visual attention (encoder-decoder attention).

Computes, for encoder_out X (B=256, P=196, E=2048), decoder_hidden h (B, D=512):
    att1 = X @ W_enc + b_enc                      (B, P, A=512)
    att2 = h @ W_dec + b_dec                      (B, 1, A)
    e    = relu(att1 + att2) @ W_att + b_att      (B, P)
    alpha = softmax(e, axis=-1)                   (B, P)
    z    = sum_p alpha[b,p] * X[b,p,:]            (B, E)
returns (z, alpha).

Strategy: data-parallel over batch across 8 NeuronCores (32 batches each).
Per core, batches are processed in groups of 4 (784 rows), with the row
(batch*pixel) dimension flattened and tiled by 128.  The heavy matmul runs in
bf16 with a "transposed activations" dataflow: X is cast to bf16 on load
(SWDGE cast-DMA), transposed on-chip via the DMA xbar so the contraction dim
(enc) lies on partitions, and att1^T (att on partitions, rows on free dim) is
accumulated in PSUM.  relu(att1+att2) is a fused vector-engine
tensor_scalar(add, max) per batch segment with att2^T as a per-partition
scalar; the scalar score e is a K-contraction matmul with W_att; softmax
runs batched (4,196) with a fused exp+sum activation; z is computed as a
rank-4 matmul per row tile against a mask-selected alpha matrix (alpha
scattered to flat-row layout via a second tiny xbar transpose).  b_enc/b_dec
fold into att2^T as K=1 rank-1 matmuls; b_att is dropped (softmax is
shift-invariant).  The z phase is software-pipelined one group behind so the
softmax round-trip hides under the next group's matmuls.
"""

import sys

sys.path.insert(0, "/opt/trn_rl_repo")

import numpy as np
from contextlib import ExitStack

import concourse.bass as bass
import concourse.tile as tile
from concourse import bacc, mybir
from concourse.bass_utils import run_bass_kernel_spmd

F32 = mybir.dt.float32
BF16 = mybir.dt.bfloat16
Act = mybir.ActivationFunctionType
Alu = mybir.AluOpType
Axis = mybir.AxisListType

B, P, E, D, A = 256, 196, 2048, 512, 512
NCORES = 8
BG = B // NCORES            # 32 batches per core
G = 4                       # batches per group
NGRP = BG // G              # 8 groups per core
GR = G * P                  # 784 rows per group
NT = 7                      # row tiles per group (6 x 128 + 1 x 16)
LAST = GR - 6 * 128         # 16
EK = E // 128               # 16 contraction k-tiles for the big matmul
AM = A // 128               # 4 attention-dim chunks
DK = D // 128               # 4 contraction k-tiles for att2
ZC = E // 512               # 4 enc chunks for z
BLOCKS = [(0, 512), (512, GR - 512)]  # PSUM column blocks of the 784 cols


def _segments():
    """Per block: (local batch, start col within block, ncols)."""
    segs = {0: [], 1: []}
    for bl in range(G):
        lo, hi = bl * P, (bl + 1) * P
        for blk, (c0, cn) in enumerate(BLOCKS):
            s, e_ = max(lo, c0), min(hi, c0 + cn)
            if s < e_:
                segs[blk].append((bl, s - c0, e_ - s))
    return segs


SEGS = _segments()


def build_nc(repeat=1):
    nc = bacc.Bacc("TRN2", target_bir_lowering=False, debug=False,
                   num_devices=NCORES)

    x = nc.dram_tensor("x", [BG * P, E], F32, kind="ExternalInput")
    h = nc.dram_tensor("h", [BG, D], F32, kind="ExternalInput")
    w_enc = nc.dram_tensor("w_enc", [E, A], F32, kind="ExternalInput")
    b_enc = nc.dram_tensor("b_enc", [A], F32, kind="ExternalInput")
    w_dec = nc.dram_tensor("w_dec", [D, A], F32, kind="ExternalInput")
    b_dec = nc.dram_tensor("b_dec", [A], F32, kind="ExternalInput")
    w_att = nc.dram_tensor("w_att", [A, 1], F32, kind="ExternalInput")
    b_att = nc.dram_tensor("b_att", [1], F32, kind="ExternalInput")

    z_out = nc.dram_tensor("z_out", [BG, E], F32, kind="ExternalOutput")
    alpha_out = nc.dram_tensor("alpha_out", [BG, P], F32, kind="ExternalOutput")
    alpha_scr = nc.dram_tensor("alpha_scr", [NGRP, GR], BF16)

    with tile.TileContext(nc) as tc, ExitStack() as ctx:
        consts = ctx.enter_context(tc.tile_pool(name="consts", bufs=1))
        xn_pool = ctx.enter_context(tc.tile_pool(name="xn", bufs=3))

        xn_cache = {}

        def cast_load(rep, g):
            """X rows of group g: cast-load to bf16 (flat 128-row tiles)."""
            row0 = g * GR
            xn = xn_pool.tile([128, NT, E], BF16, tag="xn",
                              name=f"xn{rep}_{g}")
            nc.gpsimd.dma_start(
                out=xn[:, 0:6, :],
                in_=x.ap()[row0:row0 + 768, :].rearrange("(t p) e -> p t e",
                                                         p=128))
            nc.gpsimd.dma_start(out=xn[0:LAST, 6, :],
                                in_=x.ap()[row0 + 768:row0 + GR, :])
            xn_cache[(rep, g)] = xn

        # ---- small weights via HWDGE raw f32 + DVE cast (keeps the Q7
        # SWDGE queue free for the big X cast stream) ----
        w_dec_sb = consts.tile([128, DK, A], BF16)
        w_attT_sb = consts.tile([128, AM], BF16)
        b_enc_row = consts.tile([1, A], BF16)
        b_dec_row = consts.tile([1, A], BF16)
        h_nat = consts.tile([BG, D], BF16)
        with tc.tile_pool(name="stage", bufs=1) as stage:
            st_wdec = stage.tile([128, DK, A], F32)
            nc.sync.dma_start(out=st_wdec,
                              in_=w_dec.ap().rearrange("(k p) a -> p k a",
                                                       p=128))
            st_watt = stage.tile([128, AM], F32)
            for m in range(AM):
                nc.sync.dma_start(out=st_watt[:, m:m + 1],
                                  in_=w_att.ap()[m * 128:(m + 1) * 128, :])
            st_rows = stage.tile([1, 2 * A], F32)
            nc.sync.dma_start(out=st_rows[:, 0:A], in_=b_enc.ap()[None, :])
            nc.sync.dma_start(out=st_rows[:, A:2 * A], in_=b_dec.ap()[None, :])
            st_h = stage.tile([BG, D], F32)
            nc.sync.dma_start(out=st_h, in_=h.ap())

            # start streaming X for the first group (Q7 free from here)
            cast_load(0, 0)

            # ---- W_enc (bf16, contraction dim on partitions, k-tiled) ----
            w_enc_sb = consts.tile([128, EK, A], BF16)
            w_enc_r = w_enc.ap().rearrange("(k p) a -> p k a", p=128)
            for kc in range(4):
                nc.gpsimd.dma_start(out=w_enc_sb[:, 4 * kc:4 * kc + 4, :],
                                    in_=w_enc_r[:, 4 * kc:4 * kc + 4, :])

            # DVE casts of the staged small weights
            nc.vector.tensor_copy(w_dec_sb, st_wdec)
            nc.vector.tensor_copy(w_attT_sb, st_watt)
            nc.vector.tensor_copy(b_enc_row, st_rows[:, 0:A])
            nc.vector.tensor_copy(b_dec_row, st_rows[:, A:2 * A])
            nc.vector.tensor_copy(h_nat, st_h)
        ones_row = consts.tile([1, GR], BF16)
        nc.vector.memset(ones_row, 1.0)

        # ---- static masks, baked into the NEFF as constants ----
        # m0[p, t, b] = 1 iff flat row 128t+p belongs to local batch b
        import ml_dtypes
        rows_idx = (128 * np.arange(NT)[None, :, None]
                    + np.arange(128)[:, None, None])
        b_idx = np.arange(G)[None, None, :]
        m0_np = ((P * b_idx <= rows_idx)
                 & (rows_idx < P * (b_idx + 1))).astype(ml_dtypes.bfloat16)
        m0_c = nc.inline_tensor(m0_np, name="m0_const")
        m0 = consts.tile([128, NT, G], BF16)
        nc.sync.dma_start(out=m0, in_=m0_c.ap())

        # ---- att2^T = (h @ W_dec + b_dec + b_enc)^T : (att on parts, batch) ----
        att2T = consts.tile([128, AM, BG], F32)
        with tc.tile_pool(name="att2ps", bufs=1, space="PSUM") as att2psum, \
             tc.tile_pool(name="prep", bufs=1) as prep:
            hT = prep.tile([128, DK, BG], BF16)
            nc.sync.dma_start(out=hT, in_=h_nat[0:BG, :], transpose=True)
            for m in range(AM):
                ps2 = att2psum.tile([128, BG], F32, tag="att2")
                for k in range(DK):
                    nc.tensor.matmul(ps2, w_dec_sb[:, k, m * 128:(m + 1) * 128],
                                     hT[:, k, :], start=(k == 0), stop=False)
                nc.tensor.matmul(ps2, b_dec_row[0:1, m * 128:(m + 1) * 128],
                                 ones_row[0:1, 0:BG], start=False, stop=False)
                nc.tensor.matmul(ps2, b_enc_row[0:1, m * 128:(m + 1) * 128],
                                 ones_row[0:1, 0:BG], start=False, stop=True)
                nc.vector.tensor_copy(att2T[:, m, :], ps2)

        # ---- main pipeline over batch groups ----
        xt_pool = ctx.enter_context(tc.tile_pool(name="xt", bufs=2))
        r_pool = ctx.enter_context(tc.tile_pool(name="r", bufs=3))
        small = ctx.enter_context(tc.tile_pool(name="small", bufs=2))
        att1_ps = ctx.enter_context(tc.tile_pool(name="att1ps", bufs=2,
                                                 space="PSUM"))
        e_psp = ctx.enter_context(tc.tile_pool(name="eps", bufs=1,
                                               space="PSUM"))
        z_psp = ctx.enter_context(tc.tile_pool(name="zps", bufs=1,
                                               space="PSUM"))

        def z_phase(g, xn, amask):
            """z = masked-alpha^T @ X (contraction over rows, per tile)."""
            z_ps = [z_psp.tile([G, 512], F32, tag=f"z{c}", name=f"z_ps{c}")
                    for c in range(ZC)]
            for t in range(NT):
                rows = 128 if t < 6 else LAST
                for c in range(ZC):
                    nc.tensor.matmul(z_ps[c], amask[0:rows, t, :],
                                     xn[0:rows, t, c * 512:(c + 1) * 512],
                                     start=(t == 0), stop=(t == NT - 1))
            z_sb = small.tile([G, E], F32, tag="zsb", name="z_sb")
            for c in range(ZC):
                nc.vector.tensor_copy(z_sb[:, c * 512:(c + 1) * 512], z_ps[c])
            nc.sync.dma_start(out=z_out.ap()[g * G:(g + 1) * G, :], in_=z_sb)

        pending = {}
        for rep in range(repeat):
          for g in range(NGRP):
            xn = xn_cache.pop((rep, g))
            xT = xt_pool.tile([128, EK, GR], BF16, tag="xT")
            for t in range(NT):
                rows = 128 if t < 6 else LAST
                nc.sync.dma_start(out=xT[:, :, t * 128:t * 128 + rows],
                                  in_=xn[0:rows, t, :], transpose=True)
            if g + 1 < NGRP:
                cast_load(rep, g + 1)
            elif rep + 1 < repeat:
                cast_load(rep + 1, 0)

            # att1^T chunks + relu(+att2 bias) + e accumulation
            e_ps = e_psp.tile([1, 1024], F32, tag="e")
            for m in range(AM):
                r_m = r_pool.tile([128, GR], BF16, tag="R")
                for blk, (c0, cn) in enumerate(BLOCKS):
                    ps = att1_ps.tile([128, 512], F32, tag="att1")
                    for k in range(EK):
                        nc.tensor.matmul(ps[:, 0:cn],
                                         w_enc_sb[:, k, m * 128:(m + 1) * 128],
                                         xT[:, k, c0:c0 + cn],
                                         start=(k == 0), stop=(k == EK - 1))
                    for (bl, s0, sn) in SEGS[blk]:
                        nc.vector.tensor_scalar(
                            r_m[:, c0 + s0:c0 + s0 + sn], ps[:, s0:s0 + sn],
                            att2T[:, m, g * G + bl:g * G + bl + 1], 0.0,
                            Alu.add, Alu.max)
                    nc.tensor.matmul(e_ps[:, c0:c0 + cn],
                                     w_attT_sb[:, m:m + 1], r_m[:, c0:c0 + cn],
                                     start=(m == 0), stop=(m == AM - 1))
            # (b_att is a uniform shift of e; softmax is shift-invariant, so
            # it never affects alpha or z and is deliberately dropped.)

            # e (1,784) -> (4,196): same linear byte order, one SBUF-SBUF DMA
            e_row = small.tile([1, GR], F32, tag="erow")
            nc.vector.tensor_copy(e_row[:, 0:512], e_ps[:, 0:512])
            nc.vector.tensor_copy(e_row[:, 512:GR], e_ps[:, 512:GR])
            e_bt = small.tile([G, P], F32, tag="ebt")
            nc.sync.dma_start(out=e_bt, in_=e_row)
            mneg = small.tile([G, 1], F32, tag="mneg")
            nc.vector.tensor_reduce(mneg, e_bt, axis=Axis.X, op=Alu.max,
                                    negate=True)
            pexp = small.tile([G, P], F32, tag="pexp")
            ssum = small.tile([G, 1], F32, tag="ssum")
            nc.scalar.activation(pexp, e_bt, Act.Exp, bias=mneg[:, 0:1],
                                 scale=1.0, accum_out=ssum)
            rsum = small.tile([G, 1], F32, tag="rsum")
            nc.vector.reciprocal(rsum, ssum)
            alpha_g = small.tile([G, P], F32, tag="alphag")
            nc.vector.tensor_scalar_mul(alpha_g, pexp, rsum[:, 0:1])
            nc.sync.dma_start(out=alpha_out.ap()[g * G:(g + 1) * G, :],
                              in_=alpha_g)

            # alpha -> flat-row-retiled bf16 via an all-HWDGE DRAM bounce
            a_bf = small.tile([G, P], BF16, tag="abf")
            nc.vector.tensor_copy(a_bf, alpha_g)
            nc.sync.dma_start(out=alpha_scr.ap()[g:g + 1, :].rearrange(
                "a (b p) -> a b p", b=G), in_=a_bf)
            al_nat = small.tile([16, 128], BF16, tag="alnat")
            nc.vector.memset(al_nat, 0.0)
            nc.sync.dma_start(
                out=al_nat[0:6, :],
                in_=alpha_scr.ap()[g][0:768].rearrange("(t q) -> t q", q=128))
            nc.sync.dma_start(out=al_nat[6:7, 0:LAST],
                              in_=alpha_scr.ap()[g][768:GR][None, :])
            alT = small.tile([128, 16], BF16, tag="alT")
            nc.sync.dma_start(out=alT, in_=al_nat, transpose=True)
            amask = small.tile([128, NT, G], BF16, tag="amask")
            alT_b = bass.AP(tensor=alT.tensor, offset=alT[:, 0:NT].offset,
                            ap=[alT[:, 0:NT].ap[0], alT[:, 0:NT].ap[1],
                                [0, G]])
            nc.vector.tensor_mul(amask, alT_b, m0)

            # defer z by one group so the softmax round-trip hides behind
            # the next group's matmuls
            gi = rep * NGRP + g
            pending[gi] = (g, xn, amask)
            if gi >= 1:
                pg, pxn, pam = pending.pop(gi - 1)
                z_phase(pg, pxn, pam)
        last = repeat * NGRP - 1
        pg, pxn, pam = pending.pop(last)
        z_phase(pg, pxn, pam)

    nc.compile()
    return nc


_NC_CACHE = {}


def _get_nc():
    if "nc" not in _NC_CACHE:
        _NC_CACHE["nc"] = build_nc()
    return _NC_CACHE["nc"]


def _in_maps(encoder_out, decoder_hidden, W_enc, b_enc, W_dec, b_dec, W_att,
             b_att):
    x = np.ascontiguousarray(np.asarray(encoder_out, dtype=np.float32))
    h = np.ascontiguousarray(np.asarray(decoder_hidden, dtype=np.float32))
    shared = {
        "w_enc": np.ascontiguousarray(np.asarray(W_enc, np.float32)),
        "b_enc": np.ascontiguousarray(np.asarray(b_enc, np.float32)),
        "w_dec": np.ascontiguousarray(np.asarray(W_dec, np.float32)),
        "b_dec": np.ascontiguousarray(np.asarray(b_dec, np.float32)),
        "w_att": np.ascontiguousarray(np.asarray(W_att, np.float32)),
        "b_att": np.ascontiguousarray(np.asarray(b_att, np.float32)),
    }
    maps = []
    for i in range(NCORES):
        maps.append({
            "x": np.ascontiguousarray(
                x[i * BG:(i + 1) * BG].reshape(BG * P, E)),
            "h": np.ascontiguousarray(h[i * BG:(i + 1) * BG]),
            **shared,
        })
    return maps


def kernel(encoder_out, decoder_hidden, W_enc, b_enc, W_dec, b_dec, W_att,
           b_att, _trace=False):
    nc = _get_nc()
    maps = _in_maps(encoder_out, decoder_hidden, W_enc, b_enc, W_dec, b_dec,
                    W_att, b_att)
    res = run_bass_kernel_spmd(nc, maps, core_ids=list(range(NCORES)),
                               trace=_trace)
    z = np.concatenate([np.asarray(res.results[i]["z_out"])
                        for i in range(NCORES)], axis=0)
    alpha = np.concatenate([np.asarray(res.results[i]["alpha_out"])
                            for i in range(NCORES)], axis=0)
    if _trace:
        kernel._last_exec_time_ns = res.exec_time_ns
        kernel._last_results = res
    return (z.astype(np.float32), alpha.astype(np.float32))
